# revision 9
# baseline (speedup 1.0000x reference)
"""Multi-head attention (S=2048, B=4, H=1024, NH=16) on 8 Trainium2 NeuronCores.

Sharding: each core handles 2 batches x 4 heads (batch pairs balanced by
valid length; tensor-parallel over heads). Within a core everything is bf16
matmul / fp32 accumulate:
  1. q,k projected d-major (qT/kT: [dims, seq]), v seq-major ([seq, dims])
  2. scoresT[k,q] per head-pair via row-tiled matmuls into one 2-bank tile
  3. mask+scale+exp fused on ScalarE (per-partition bias; PAD keys -> exp 0)
  4. PV col-tiled accumulates attnT; row-sums Z via col-tiled ones-matmuls
  5. attnT normalized by keepq/Z (reciprocal + partition_broadcast + mul)
  6. Wo projection -> yT partial [H, S]; host sums 4 partials/batch, transposes
Padded key/query chunks are skipped at 128/512 granularity; because the
compiled program is shared SPMD, trip counts are the per-slot max over cores
and any over-computation is exactly zeroed by the masking.
"""
import sys

if "/opt/trn_rl_repo" not in sys.path:
    sys.path.insert(0, "/opt/trn_rl_repo")

import math
import os
from itertools import permutations

import ml_dtypes
import numpy as np

import concourse.bass as bass
import concourse.mybir as mybir
import concourse.tile as tile
from concourse import bacc
from concourse.bass_utils import run_bass_kernel_spmd

S, B, H, NH, DK = 2048, 4, 1024, 16, 64
N_CORES = 8
BF16 = mybir.dt.bfloat16
F32 = mybir.dt.float32
NPBF16 = ml_dtypes.bfloat16
MASK_BIAS = -30000.0

_prog_cache: dict = {}


def _build_program(NQ, NK):
    """One SPMD program. NQ[s]: 512-wide q chunks, NK[s]: 128-wide k chunks
    per batch-slot s in {0,1}."""
    NSCK = [(nk * 128 + 511) // 512 for nk in NK]  # 512-wide seq chunks for k-proj
    nc = bacc.Bacc("TRN2", target_bir_lowering=False, debug=False,
                   num_devices=N_CORES)

    d_in = {}
    for s in range(2):
        d_in[f"qT{s}"] = nc.dram_tensor(f"qT{s}", [H, S], BF16, kind="ExternalInput")
        d_in[f"kT{s}"] = nc.dram_tensor(f"kT{s}", [H, S], BF16, kind="ExternalInput")
        d_in[f"vT{s}"] = nc.dram_tensor(f"vT{s}", [H, S], BF16, kind="ExternalInput")
        d_in[f"mb{s}"] = nc.dram_tensor(f"mb{s}", [128, 16], F32, kind="ExternalInput")
        d_in[f"kq{s}"] = nc.dram_tensor(f"kq{s}", [4, 512], F32, kind="ExternalInput")
    d_in["wqT"] = nc.dram_tensor("wqT", [H, 256], BF16, kind="ExternalInput")
    d_in["wkT"] = nc.dram_tensor("wkT", [H, 256], BF16, kind="ExternalInput")
    d_in["wvT"] = nc.dram_tensor("wvT", [H, 256], BF16, kind="ExternalInput")
    d_in["woT"] = nc.dram_tensor("woT", [256, H], BF16, kind="ExternalInput")
    d_out = [nc.dram_tensor(f"y{s}", [H, S], BF16, kind="ExternalOutput")
             for s in range(2)]
    DBG = os.environ.get("KERNEL_DEBUG_DUMP", "0") == "1"
    d_dbg = {}
    if DBG:
        for nm, shp, dt in [("dbg_qT", [128, S], BF16), ("dbg_kT", [128, S], BF16),
                            ("dbg_v", [128, 256], BF16), ("dbg_pr", [128, 1024], BF16),
                            ("dbg_z", [128, 512], F32), ("dbg_rz", [128, 512], F32),
                            ("dbg_zrep", [128, 512], F32), ("dbg_ab", [128, 512], F32),
                            ("dbg_at", [128, 512], F32)]:
            d_dbg[nm] = nc.dram_tensor(nm, shp, dt, kind="ExternalOutput")

    with tile.TileContext(nc) as tc:
        with tc.tile_pool(name="wpool", bufs=1) as wpool, \
             tc.tile_pool(name="inp", bufs=3) as inp, \
             tc.tile_pool(name="persist", bufs=1) as persist, \
             tc.tile_pool(name="probs", bufs=4) as probsp, \
             tc.tile_pool(name="small", bufs=4) as small, \
             tc.tile_pool(name="att", bufs=4) as attp, \
             tc.tile_pool(name="yst", bufs=4) as ystp:

            # --- constants / weights ---
            wq = [wpool.tile([128, 256], BF16, name=f"wq{i}", tag=f"wq{i}")
                  for i in range(8)]
            wk = [wpool.tile([128, 256], BF16, name=f"wk{i}", tag=f"wk{i}")
                  for i in range(8)]
            wv = [wpool.tile([128, 256], BF16, name=f"wv{i}", tag=f"wv{i}")
                  for i in range(8)]
            wo = [wpool.tile([128, H], BF16, name=f"wo{j}", tag=f"wo{j}")
                  for j in range(2)]
            for i in range(8):
                nc.sync.dma_start(out=wq[i][:], in_=d_in["wqT"].ap()[i * 128:(i + 1) * 128, :])
                nc.sync.dma_start(out=wk[i][:], in_=d_in["wkT"].ap()[i * 128:(i + 1) * 128, :])
                nc.sync.dma_start(out=wv[i][:], in_=d_in["wvT"].ap()[i * 128:(i + 1) * 128, :])
            for j in range(2):
                nc.sync.dma_start(out=wo[j][:], in_=d_in["woT"].ap()[j * 128:(j + 1) * 128, :])
            mb = [wpool.tile([128, 16], F32, name=f"mbt{s}", tag=f"mbt{s}") for s in range(2)]
            for s in range(2):
                nc.sync.dma_start(out=mb[s][:], in_=d_in[f"mb{s}"].ap())
            ones = wpool.tile([128, 1], BF16, name="ones", tag="ones")
            nc.vector.memset(ones[:], 1.0)

            # --- persistent projection outputs ---
            qTp = [[persist.tile([128, S], BF16, name=f"qTp{s}_{p}", tag=f"qTp{s}_{p}")
                    for p in range(2)] for s in range(2)]
            kTp = [[persist.tile([128, S], BF16, name=f"kTp{s}_{p}", tag=f"kTp{s}_{p}")
                    for p in range(2)] for s in range(2)]
            vp = [[persist.tile([128, 256], BF16, name=f"vp{s}_{st}", tag=f"vp{s}_{st}")
                   for st in range(NK[s])] for s in range(2)]

            # --- projections (own PSUM pool, released before attention) ---
            with tc.tile_pool(name="pproj", bufs=1, space="PSUM") as pproj:
                for s in range(2):
                    # q and k: weights stationary, inputs streamed (ic outer)
                    for kind, wts, dname, nsc, outtiles in (
                            ("q", wq, f"qT{s}", NQ[s], qTp[s]),
                            ("k", wk, f"kT{s}", NSCK[s], kTp[s])):
                        ps = [[pproj.tile([128, 512], F32,
                                          name=f"pj{kind}{s}_{ft}_{sc}",
                                          tag=f"pj_{ft}_{sc}")
                               for sc in range(nsc)] for ft in range(2)]
                        for ic in range(8):
                            it = inp.tile([128, nsc * 512], BF16,
                                          name=f"in{kind}{s}_{ic}", tag="inp")
                            nc.sync.dma_start(out=it[:], in_=d_in[dname].ap()[ic * 128:(ic + 1) * 128, 0:nsc * 512])
                            for ft in range(2):
                                for sc in range(nsc):
                                    nc.tensor.matmul(
                                        out=ps[ft][sc][:],
                                        lhsT=wts[ic][:, ft * 128:(ft + 1) * 128],
                                        rhs=it[:, sc * 512:(sc + 1) * 512],
                                        start=(ic == 0), stop=(ic == 7))
                        for ft in range(2):
                            for sc in range(nsc):
                                nc.vector.tensor_copy(
                                    outtiles[ft][:, sc * 512:(sc + 1) * 512],
                                    ps[ft][sc][:])
                    # v: valueT stationary per (ic, seqtile), W streamed
                    for st0 in range(0, NK[s], 8):
                        sts = range(st0, min(st0 + 8, NK[s]))
                        psv = {st: pproj.tile([128, 256], F32,
                                              name=f"pjv{s}_{st}",
                                              tag=f"pj_{(st - st0) // 4}_{(st - st0) % 4}")
                               for st in sts}
                        for ic in range(8):
                            it = inp.tile([128, 1024], BF16,
                                          name=f"inv{s}_{st0}_{ic}", tag="inp")
                            nc.sync.dma_start(
                                out=it[:, 0:len(sts) * 128],
                                in_=d_in[f"vT{s}"].ap()[ic * 128:(ic + 1) * 128,
                                                        st0 * 128:(st0 + len(sts)) * 128])
                            for st in sts:
                                nc.tensor.matmul(
                                    out=psv[st][:],
                                    lhsT=it[:, (st - st0) * 128:(st - st0 + 1) * 128],
                                    rhs=wv[ic][:, :],
                                    start=(ic == 0), stop=(ic == 7))
                        for st in sts:
                            nc.vector.tensor_copy(vp[s][st][:], psv[st][:])

            # --- attention + output projection ---
            with tc.tile_pool(name="psc", bufs=2, space="PSUM") as psc, \
                 tc.tile_pool(name="pat", bufs=2, space="PSUM") as pat, \
                 tc.tile_pool(name="pz", bufs=1, space="PSUM") as pz, \
                 tc.tile_pool(name="pwo", bufs=1, space="PSUM") as pwo:
                for s in range(2):
                    for qc in range(NQ[s]):
                        kqr = small.tile([128, 512], F32, name=f"kqr{s}_{qc}",
                                         tag="kqr")
                        nc.sync.dma_start(
                            out=kqr[:],
                            in_=bass.AP(tensor=d_in[f"kq{s}"], offset=qc * 512,
                                        ap=[[0, 128], [1, 512]]))
                        zps = pz.tile([128, 512], F32, name=f"z{s}_{qc}", tag="z")
                        att_sb = []
                        for p in range(2):
                            attn = pat.tile([128, 512], F32,
                                            name=f"at{s}_{qc}_{p}", tag="at")
                            for kc in range(NK[s]):
                                first, last = kc == 0, kc == NK[s] - 1
                                sc_ps = psc.tile([128, 1024], F32,
                                                 name=f"s{s}_{qc}_{p}_{kc}",
                                                 tag="sc")
                                pr = probsp.tile([128, 1024], BF16,
                                                 name=f"pr{s}_{qc}_{p}_{kc}",
                                                 tag="pr")
                                for hh in range(2):
                                    hsl = slice(hh * 64, hh * 64 + 64)
                                    qsl = slice(hh * 512, hh * 512 + 512)
                                    nc.tensor.matmul(
                                        out=sc_ps[:, qsl],
                                        lhsT=kTp[s][p][hsl, kc * 128:(kc + 1) * 128],
                                        rhs=qTp[s][p][hsl, qc * 512:(qc + 1) * 512],
                                        start=True, stop=True)
                                nc.scalar.activation(
                                    out=pr[:], in_=sc_ps[:],
                                    func=mybir.ActivationFunctionType.Exp,
                                    bias=mb[s][:, kc:kc + 1],
                                    scale=1.0 / math.sqrt(DK))
                                if DBG and s == 0 and qc == 0 and p == 0 and kc == 0:
                                    nc.gpsimd.dma_start(out=d_dbg["dbg_pr"].ap(), in_=pr[:])
                                for hh in range(2):
                                    hsl = slice(hh * 64, hh * 64 + 64)
                                    qsl = slice(hh * 512, hh * 512 + 512)
                                    nc.tensor.matmul(
                                        out=attn[hsl, :],
                                        lhsT=vp[s][kc][:, p * 128 + hh * 64:p * 128 + (hh + 1) * 64],
                                        rhs=pr[:, qsl], start=first, stop=last)
                                    strip = (2 * p + hh) * 32
                                    nc.tensor.matmul(
                                        out=zps[strip:strip + 1, :],
                                        lhsT=ones[:, :], rhs=pr[:, qsl],
                                        start=first, stop=last,
                                        tile_position=(0, strip))
                            # normalize this pair: attn * keepq/Z.
                            # partition_broadcast only works with src AND dst
                            # at physical partition 0 -> DMA each Z row down,
                            # broadcast to a full 128-row tile, multiply the
                            # matching 64-row slice.
                            rz = small.tile([128, 512], F32,
                                            name=f"rz{s}_{qc}_{p}", tag="rz")
                            ab = attp.tile([128, 512], BF16,
                                           name=f"ab{s}_{qc}_{p}", tag=f"ab{p}")
                            for hh in range(2):
                                j = (2 * p + hh) * 32
                                nc.vector.reciprocal(out=rz[j:j + 1, :],
                                                     in_=zps[j:j + 1, :])
                                nc.vector.tensor_mul(rz[j:j + 1, :],
                                                     rz[j:j + 1, :],
                                                     kqr[j:j + 1, :])
                                if j == 0:
                                    src = rz[0:1, :]
                                else:
                                    tmp = small.tile([1, 512], F32,
                                                     name=f"rzmv{s}_{qc}_{p}_{hh}",
                                                     tag=f"rzmv{hh}")
                                    nc.sync.dma_start(out=tmp[0:1, :],
                                                      in_=rz[j:j + 1, :])
                                    src = tmp[0:1, :]
                                zrh = small.tile([128, 512], F32,
                                                 name=f"zrf{s}_{qc}_{p}_{hh}",
                                                 tag=f"zrf{hh}")
                                nc.gpsimd.partition_broadcast(zrh[:, :], src)
                                hsl = slice(hh * 64, (hh + 1) * 64)
                                nc.vector.tensor_mul(ab[hsl, :], attn[hsl, :],
                                                     zrh[hsl, :])
                            att_sb.append(ab)
                            if DBG and s == 0 and qc == 0 and p == 0:
                                dtmp = small.tile([128, 512], F32, name="dtmp", tag="dtmp")
                                nc.vector.tensor_copy(dtmp[:], attn[:])
                                nc.gpsimd.dma_start(out=d_dbg["dbg_at"].ap(), in_=dtmp[:])
                                dtmp2 = small.tile([128, 512], F32, name="dtmp2", tag="dtmp2")
                                nc.vector.tensor_copy(dtmp2[:], zps[:])
                                nc.gpsimd.dma_start(out=d_dbg["dbg_z"].ap(), in_=dtmp2[:])
                                nc.gpsimd.dma_start(out=d_dbg["dbg_rz"].ap(), in_=rz[:])
                                nc.gpsimd.dma_start(out=d_dbg["dbg_zrep"].ap(), in_=zrep[:])
                                dtmp3 = small.tile([128, 512], F32, name="dtmp3", tag="dtmp3")
                                nc.vector.tensor_copy(dtmp3[:], ab[:])
                                nc.gpsimd.dma_start(out=d_dbg["dbg_ab"].ap(), in_=dtmp3[:])
                                nc.gpsimd.dma_start(out=d_dbg["dbg_qT"].ap(), in_=qTp[0][0][:])
                                nc.gpsimd.dma_start(out=d_dbg["dbg_kT"].ap(), in_=kTp[0][0][:])
                                nc.gpsimd.dma_start(out=d_dbg["dbg_v"].ap(), in_=vp[0][0][:])
                        # Wo: yT[ot, qc] = sum_j woT[j, ot].T @ attnT_j
                        for ot in range(8):
                            yps = pwo.tile([128, 512], F32,
                                           name=f"yp{s}_{qc}_{ot}", tag="y")
                            for j in range(2):
                                nc.tensor.matmul(
                                    out=yps[:],
                                    lhsT=wo[j][:, ot * 128:(ot + 1) * 128],
                                    rhs=att_sb[j][:], start=(j == 0),
                                    stop=(j == 1))
                            ysb = ystp.tile([128, 512], BF16,
                                            name=f"ysb{s}_{qc}_{ot}", tag="ysb")
                            nc.vector.tensor_copy(ysb[:], yps[:])
                            nc.gpsimd.dma_start(
                                out=d_out[s].ap()[ot * 128:(ot + 1) * 128,
                                                  qc * 512:(qc + 1) * 512],
                                in_=ysb[:])
    nc.compile()
    return nc


def _get_program(NQ, NK):
    key = (tuple(NQ), tuple(NK))
    if key not in _prog_cache:
        _prog_cache[key] = _build_program(list(NQ), list(NK))
    return _prog_cache[key]


def kernel(value, key, query, padding_mask, Wq, Wk, Wv, Wo):
    value = np.asarray(value)
    key = np.asarray(key)
    query = np.asarray(query)
    padding_mask = np.asarray(padding_mask)
    Wq, Wk, Wv, Wo = (np.asarray(a) for a in (Wq, Wk, Wv, Wo))

    lengths = (~padding_mask).sum(axis=0).astype(int)  # (B,)

    # --- batch pairing: assign batches to (group, slot) minimizing baked work ---
    def cost(assign):
        tot = 0
        for sl in range(2):
            nq = max((int(lengths[assign[g][sl]]) + 511) // 512 for g in range(2))
            nk = max((int(lengths[assign[g][sl]]) + 127) // 128 for g in range(2))
            tot += nq * nk
        return tot

    best = None
    for perm in permutations(range(B)):
        a = ((perm[0], perm[1]), (perm[2], perm[3]))
        c = cost(a)
        if best is None or c < best[0]:
            best = (c, a)
    assign = best[1]
    NQ = [max((int(lengths[assign[g][sl]]) + 511) // 512 for g in range(2))
          for sl in range(2)]
    NK = [max((int(lengths[assign[g][sl]]) + 127) // 128 for g in range(2))
          for sl in range(2)]

    nc = _get_program(NQ, NK)

    # --- per-core inputs ---
    WqT = np.ascontiguousarray(Wq.T).astype(NPBF16)
    WkT = np.ascontiguousarray(Wk.T).astype(NPBF16)
    WvT = np.ascontiguousarray(Wv.T).astype(NPBF16)
    WoT = np.ascontiguousarray(Wo.T).astype(NPBF16)

    batch_qT, batch_kT, batch_vT, batch_mb, batch_kq = {}, {}, {}, {}, {}
    for b in range(B):
        batch_qT[b] = np.ascontiguousarray(query[:, b, :].T).astype(NPBF16)
        batch_kT[b] = np.ascontiguousarray(key[:, b, :].T).astype(NPBF16)
        batch_vT[b] = np.ascontiguousarray(value[:, b, :].T).astype(NPBF16)
        kpos = np.arange(S).reshape(16, 128)  # [kchunk, kpos]
        mbv = np.where(kpos >= lengths[b], np.float32(MASK_BIAS), np.float32(0.0))
        batch_mb[b] = np.ascontiguousarray(mbv.T).astype(np.float32)  # [128, 16]
        batch_kq[b] = (np.arange(S).reshape(4, 512) < lengths[b]).astype(np.float32)

    in_maps = []
    for c in range(N_CORES):
        g, hq = c // 4, c % 4
        f0 = hq * 256
        m = {
            "wqT": np.ascontiguousarray(WqT[:, f0:f0 + 256]),
            "wkT": np.ascontiguousarray(WkT[:, f0:f0 + 256]),
            "wvT": np.ascontiguousarray(WvT[:, f0:f0 + 256]),
            "woT": np.ascontiguousarray(WoT[f0:f0 + 256, :]),
        }
        for sl in range(2):
            b = assign[g][sl]
            m[f"qT{sl}"] = batch_qT[b]
            m[f"kT{sl}"] = batch_kT[b]
            m[f"vT{sl}"] = batch_vT[b]
            m[f"mb{sl}"] = batch_mb[b]
            m[f"kq{sl}"] = batch_kq[b]
        in_maps.append(m)

    res = run_bass_kernel_spmd(nc, in_maps, list(range(N_CORES)))

    # --- gather: sum 4 head-quad partials per batch, transpose ---
    out = np.zeros((S, B, H), dtype=np.float32)
    for g in range(2):
        for sl in range(2):
            b = assign[g][sl]
            acc = np.zeros((H, S), dtype=np.float32)
            for hq in range(4):
                c = g * 4 + hq
                acc += res.results[c][f"y{sl}"].astype(np.float32)
            out[:, b, :] = acc.T
    return out


# revision 19
# speedup vs baseline: 1.8482x; 1.8482x over previous
"""Multi-head attention (S=2048, B=4, H=1024, NH=16) on 8 Trainium2 NeuronCores.

Sharding: each core handles 2 batches x 4 heads (batch pairs balanced by
valid length; tensor-parallel over heads). Within a core everything is bf16
matmul / fp32 accumulate:
  1. q,k projected d-major (qT/kT: [dims, seq]), v seq-major ([seq, dims])
  2. scoresT[k,q] per head-pair via row-tiled matmuls into one 2-bank tile
  3. mask+scale+exp fused on ScalarE (per-partition bias; PAD keys -> exp 0)
  4. PV col-tiled accumulates attnT; Z row-sums land replicated across each
     head's 64 partitions via an all-ones stationary matmul
  5. attnT normalized by 1/Z (fast approx reciprocal); keepq applied at the
     Wo output multiply
  6. Wo projection -> yT partial [H, S]; host sums 4 partials/batch, transposes
Emission order: proj(slot0) -> attention(slot0) -> proj(slot1) ->
attention(slot1), so slot1's input DMA + projection matmuls hide inside the
ScalarE-paced attention of slot0. The attention kc-loop is software-pipelined
(scores(kc+1) emitted before PV(kc)). Padded key/query chunks are skipped at
128/512 granularity; trip counts are the per-slot max over cores (shared SPMD
program) and any over-computation is exactly zeroed by the masking.
"""
import sys

if "/opt/trn_rl_repo" not in sys.path:
    sys.path.insert(0, "/opt/trn_rl_repo")

import math
from itertools import permutations

import ml_dtypes
import numpy as np

import concourse.bass as bass
import concourse.mybir as mybir
import concourse.tile as tile
from concourse import bacc
from concourse.bass_utils import run_bass_kernel_spmd

S, B, H, NH, DK = 2048, 4, 1024, 16, 64
N_CORES = 8
BF16 = mybir.dt.bfloat16
F32 = mybir.dt.float32
NPBF16 = ml_dtypes.bfloat16
MASK_BIAS = -30000.0

_prog_cache: dict = {}


def _build_program(NQ, NK):
    """One SPMD program. NQ[s]: 512-wide q chunks, NK[s]: 128-wide k chunks
    per batch-slot s in {0,1}. Slot 0 should be the smaller workload."""
    NSCK = [(nk * 128 + 511) // 512 for nk in NK]
    nc = bacc.Bacc("TRN2", target_bir_lowering=False, debug=False,
                   num_devices=N_CORES)

    d_in = {}
    for s in range(2):
        d_in[f"qT{s}"] = nc.dram_tensor(f"qT{s}", [H, S], BF16, kind="ExternalInput")
        d_in[f"kT{s}"] = nc.dram_tensor(f"kT{s}", [H, S], BF16, kind="ExternalInput")
        d_in[f"vT{s}"] = nc.dram_tensor(f"vT{s}", [H, S], BF16, kind="ExternalInput")
        d_in[f"mb{s}"] = nc.dram_tensor(f"mb{s}", [128, 16], F32, kind="ExternalInput")
        d_in[f"kq{s}"] = nc.dram_tensor(f"kq{s}", [4, 512], F32, kind="ExternalInput")
    d_in["wqT"] = nc.dram_tensor("wqT", [H, 256], BF16, kind="ExternalInput")
    d_in["wkT"] = nc.dram_tensor("wkT", [H, 256], BF16, kind="ExternalInput")
    d_in["wvT"] = nc.dram_tensor("wvT", [H, 256], BF16, kind="ExternalInput")
    d_in["woT"] = nc.dram_tensor("woT", [256, H], BF16, kind="ExternalInput")
    d_out = [nc.dram_tensor(f"y{s}", [H, S], BF16, kind="ExternalOutput")
             for s in range(2)]

    with tile.TileContext(nc) as tc:
        with tc.tile_pool(name="wpool", bufs=1) as wpool, \
             tc.tile_pool(name="inp", bufs=3) as inp, \
             tc.tile_pool(name="in8", bufs=8) as in8, \
             tc.tile_pool(name="persist", bufs=1) as persist, \
             tc.tile_pool(name="probs", bufs=4) as probsp, \
             tc.tile_pool(name="small", bufs=4) as small, \
             tc.tile_pool(name="att", bufs=4) as attp, \
             tc.tile_pool(name="yst", bufs=4) as ystp:

            # --- constants / weights ---
            wq = [wpool.tile([128, 256], BF16, name=f"wq{i}", tag=f"wq{i}")
                  for i in range(8)]
            wk = [wpool.tile([128, 256], BF16, name=f"wk{i}", tag=f"wk{i}")
                  for i in range(8)]
            wv = [wpool.tile([128, 256], BF16, name=f"wv{i}", tag=f"wv{i}")
                  for i in range(8)]
            wo = [wpool.tile([128, H], BF16, name=f"wo{j}", tag=f"wo{j}")
                  for j in range(2)]
            for i in range(8):
                nc.sync.dma_start(out=wq[i][:], in_=d_in["wqT"].ap()[i * 128:(i + 1) * 128, :])
                nc.sync.dma_start(out=wk[i][:], in_=d_in["wkT"].ap()[i * 128:(i + 1) * 128, :])
                nc.sync.dma_start(out=wv[i][:], in_=d_in["wvT"].ap()[i * 128:(i + 1) * 128, :])
            for j in range(2):
                nc.sync.dma_start(out=wo[j][:], in_=d_in["woT"].ap()[j * 128:(j + 1) * 128, :])
            mb = [wpool.tile([128, 16], F32, name=f"mbt{s}", tag=f"mbt{s}")
                  for s in range(2)]
            for s in range(2):
                nc.sync.dma_start(out=mb[s][:], in_=d_in[f"mb{s}"].ap())
            ones = wpool.tile([128, 64], BF16, name="ones", tag="ones")
            nc.vector.memset(ones[:], 1.0)

            # --- persistent projection outputs ---
            qTp = [[persist.tile([128, S], BF16, name=f"qTp{s}_{p}", tag=f"qTp{s}_{p}")
                    for p in range(2)] for s in range(2)]
            kTp = [[persist.tile([128, S], BF16, name=f"kTp{s}_{p}", tag=f"kTp{s}_{p}")
                    for p in range(2)] for s in range(2)]
            vp = [[persist.tile([128, 256], BF16, name=f"vp{s}_{st}", tag=f"vp{s}_{st}")
                   for st in range(NK[s])] for s in range(2)]

            def emit_proj_streamed(s, pool):
                """ic-outer projections with streamed inputs; needs up to
                2*max(NQ,NSCK) concurrent PSUM banks from `pool`."""
                for kind, wts, dname, nsc, outtiles in (
                        ("q", wq, f"qT{s}", NQ[s], qTp[s]),
                        ("k", wk, f"kT{s}", NSCK[s], kTp[s])):
                    ps = [[pool.tile([128, 512], F32,
                                     name=f"pj{kind}{s}_{ft}_{sc}",
                                     tag=f"pj_{ft}_{sc}")
                           for sc in range(nsc)] for ft in range(2)]
                    for ic in range(8):
                        it = inp.tile([128, nsc * 512], BF16,
                                      name=f"in{kind}{s}_{ic}", tag="inp")
                        nc.sync.dma_start(
                            out=it[:],
                            in_=d_in[dname].ap()[ic * 128:(ic + 1) * 128, 0:nsc * 512])
                        for ft in range(2):
                            for sc in range(nsc):
                                nc.tensor.matmul(
                                    out=ps[ft][sc][:],
                                    lhsT=wts[ic][:, ft * 128:(ft + 1) * 128],
                                    rhs=it[:, sc * 512:(sc + 1) * 512],
                                    start=(ic == 0), stop=(ic == 7))
                    for ft in range(2):
                        for sc in range(nsc):
                            if kind == "q":
                                nc.vector.tensor_copy(
                                    outtiles[ft][:, sc * 512:(sc + 1) * 512],
                                    ps[ft][sc][:])
                            else:
                                nc.scalar.copy(
                                    outtiles[ft][:, sc * 512:(sc + 1) * 512],
                                    ps[ft][sc][:])
                for st0 in range(0, NK[s], 8):
                    sts = range(st0, min(st0 + 8, NK[s]))
                    psv = {st: pool.tile([128, 256], F32, name=f"pjv{s}_{st}",
                                         tag=f"pj_{(st - st0) // 4}_{(st - st0) % 4}")
                           for st in sts}
                    for ic in range(8):
                        it = inp.tile([128, 1024], BF16,
                                      name=f"inv{s}_{st0}_{ic}", tag="inp")
                        nc.sync.dma_start(
                            out=it[:, 0:len(sts) * 128],
                            in_=d_in[f"vT{s}"].ap()[ic * 128:(ic + 1) * 128,
                                                    st0 * 128:(st0 + len(sts)) * 128])
                        for st in sts:
                            nc.tensor.matmul(
                                out=psv[st][:],
                                lhsT=it[:, (st - st0) * 128:(st - st0 + 1) * 128],
                                rhs=wv[ic][:, :],
                                start=(ic == 0), stop=(ic == 7))
                    for st in sts:
                        if st % 2:
                            nc.scalar.copy(vp[s][st][:], psv[st][:])
                        else:
                            nc.vector.tensor_copy(vp[s][st][:], psv[st][:])

            def emit_proj_resident(s, pool):
                """(group)-outer ic-inner projections with resident inputs;
                uses only the shared 2-slot PSUM `pool` (tag 'sc'), so it can
                interleave with a running attention phase."""
                # q then k then v reuse the same 8 input slots
                for kind, wts, dname, nsc, outtiles in (
                        ("q", wq, f"qT{s}", NQ[s], qTp[s]),
                        ("k", wk, f"kT{s}", NSCK[s], kTp[s])):
                    tiles = []
                    for ic in range(8):
                        it = in8.tile([128, nsc * 512], BF16,
                                      name=f"r{kind}{s}_{ic}", tag="in8")
                        nc.sync.dma_start(
                            out=it[:],
                            in_=d_in[dname].ap()[ic * 128:(ic + 1) * 128, 0:nsc * 512])
                        tiles.append(it)
                    for ft in range(2):
                        for sc in range(nsc):
                            pj = pool.tile([128, 1024], F32,
                                           name=f"rpj{kind}{s}_{ft}_{sc}",
                                           tag="sc")
                            for ic in range(8):
                                nc.tensor.matmul(
                                    out=pj[:, 0:512],
                                    lhsT=wts[ic][:, ft * 128:(ft + 1) * 128],
                                    rhs=tiles[ic][:, sc * 512:(sc + 1) * 512],
                                    start=(ic == 0), stop=(ic == 7))
                            if kind == "q":
                                nc.vector.tensor_copy(
                                    outtiles[ft][:, sc * 512:(sc + 1) * 512],
                                    pj[:, 0:512])
                            else:
                                nc.scalar.copy(
                                    outtiles[ft][:, sc * 512:(sc + 1) * 512],
                                    pj[:, 0:512])
                tiles = []
                for ic in range(8):
                    it = in8.tile([128, NK[s] * 128], BF16,
                                  name=f"rv{s}_{ic}", tag="in8")
                    nc.sync.dma_start(
                        out=it[:],
                        in_=d_in[f"vT{s}"].ap()[ic * 128:(ic + 1) * 128,
                                                0:NK[s] * 128])
                    tiles.append(it)
                for st in range(NK[s]):
                    pj = pool.tile([128, 1024], F32, name=f"rpjv{s}_{st}",
                                   tag="sc")
                    for ic in range(8):
                        nc.tensor.matmul(
                            out=pj[:, 0:256],
                            lhsT=tiles[ic][:, st * 128:(st + 1) * 128],
                            rhs=wv[ic][:, :],
                            start=(ic == 0), stop=(ic == 7))
                    if st % 2:
                        nc.scalar.copy(vp[s][st][:], pj[:, 0:256])
                    else:
                        nc.vector.tensor_copy(vp[s][st][:], pj[:, 0:256])

            def emit_attention(s, psc, pat, pzy):
                for qc in range(NQ[s]):
                    kqr = small.tile([128, 512], F32, name=f"kqr{s}_{qc}",
                                     tag="kqr")
                    nc.sync.dma_start(
                        out=kqr[:],
                        in_=bass.AP(tensor=d_in[f"kq{s}"], offset=qc * 512,
                                    ap=[[0, 128], [1, 512]]))
                    att_sb = []
                    for p in range(2):
                        attn = pat.tile([128, 512], F32,
                                        name=f"at{s}_{qc}_{p}", tag="at")
                        zps = pzy.tile([128, 512], F32,
                                       name=f"z{s}_{qc}_{p}", tag="zy")

                        # software-pipelined: scores(kc+1) emitted before
                        # PV(kc) so the in-order PE queue never stalls on exp
                        def emit_scores(kc):
                            sc_ps = psc.tile([128, 1024], F32,
                                             name=f"s{s}_{qc}_{p}_{kc}",
                                             tag="sc")
                            pr = probsp.tile([128, 1024], BF16,
                                             name=f"pr{s}_{qc}_{p}_{kc}",
                                             tag="pr")
                            for hh in range(2):
                                hsl = slice(hh * 64, hh * 64 + 64)
                                qsl = slice(hh * 512, hh * 512 + 512)
                                nc.tensor.matmul(
                                    out=sc_ps[:, qsl],
                                    lhsT=kTp[s][p][hsl, kc * 128:(kc + 1) * 128],
                                    rhs=qTp[s][p][hsl, qc * 512:(qc + 1) * 512],
                                    start=True, stop=True)
                            nc.scalar.activation(
                                out=pr[:], in_=sc_ps[:],
                                func=mybir.ActivationFunctionType.Exp,
                                bias=mb[s][:, kc:kc + 1],
                                scale=1.0 / math.sqrt(DK))
                            return pr

                        pr_next = emit_scores(0)
                        for kc in range(NK[s]):
                            first, last = kc == 0, kc == NK[s] - 1
                            pr = pr_next
                            if not last:
                                pr_next = emit_scores(kc + 1)
                            for hh in range(2):
                                hsl = slice(hh * 64, hh * 64 + 64)
                                qsl = slice(hh * 512, hh * 512 + 512)
                                nc.tensor.matmul(
                                    out=attn[hsl, :],
                                    lhsT=vp[s][kc][:, p * 128 + hh * 64:p * 128 + (hh + 1) * 64],
                                    rhs=pr[:, qsl], start=first, stop=last)
                                nc.tensor.matmul(
                                    out=zps[hsl, :],
                                    lhsT=ones[:, :], rhs=pr[:, qsl],
                                    start=first, stop=last)
                        # normalize: attn / Z  (keepq folded into Wo stage)
                        rz = small.tile([128, 512], F32,
                                        name=f"rz{s}_{qc}_{p}", tag="rz")
                        nc.vector.reciprocal_approx_fast(out=rz[:], in_=zps[:])
                        ab = attp.tile([128, 512], BF16,
                                       name=f"ab{s}_{qc}_{p}", tag=f"ab{p}")
                        nc.vector.tensor_mul(ab[:], attn[:], rz[:])
                        att_sb.append(ab)
                    # Wo: yT[ot, qc] = sum_j woT[j, ot].T @ attnT_j
                    for ot in range(8):
                        yps = pzy.tile([128, 512], F32,
                                       name=f"yp{s}_{qc}_{ot}", tag="zy")
                        for j in range(2):
                            nc.tensor.matmul(
                                out=yps[:],
                                lhsT=wo[j][:, ot * 128:(ot + 1) * 128],
                                rhs=att_sb[j][:], start=(j == 0), stop=(j == 1))
                        ysb = ystp.tile([128, 512], BF16,
                                        name=f"ysb{s}_{qc}_{ot}", tag="ysb")
                        # multiply by keepq: zeroes PAD query columns
                        nc.vector.tensor_mul(ysb[:], yps[:], kqr[:])
                        nc.gpsimd.dma_start(
                            out=d_out[s].ap()[ot * 128:(ot + 1) * 128,
                                              qc * 512:(qc + 1) * 512],
                            in_=ysb[:])

            # slot 0 projections use the full PSUM (released afterwards)
            with tc.tile_pool(name="pproj", bufs=1, space="PSUM") as pproj:
                emit_proj_streamed(0, pproj)
            with tc.tile_pool(name="psc", bufs=2, space="PSUM") as psc, \
                 tc.tile_pool(name="pat", bufs=2, space="PSUM") as pat, \
                 tc.tile_pool(name="pzy", bufs=2, space="PSUM") as pzy:
                emit_attention(0, psc, pat, pzy)
                emit_proj_resident(1, psc)
                emit_attention(1, psc, pat, pzy)
    nc.compile()
    return nc


def _get_program(NQ, NK):
    key = (tuple(NQ), tuple(NK))
    if key not in _prog_cache:
        _prog_cache[key] = _build_program(list(NQ), list(NK))
    return _prog_cache[key]


def kernel(value, key, query, padding_mask, Wq, Wk, Wv, Wo):
    value = np.asarray(value)
    key = np.asarray(key)
    query = np.asarray(query)
    padding_mask = np.asarray(padding_mask)
    Wq, Wk, Wv, Wo = (np.asarray(a) for a in (Wq, Wk, Wv, Wo))

    lengths = (~padding_mask).sum(axis=0).astype(int)  # (B,)

    # --- batch pairing: assign batches to (group, slot) minimizing baked work ---
    def slot_counts(assign):
        nq = [max((int(lengths[assign[g][sl]]) + 511) // 512 for g in range(2))
              for sl in range(2)]
        nk = [max((int(lengths[assign[g][sl]]) + 127) // 128 for g in range(2))
              for sl in range(2)]
        return nq, nk

    best = None
    for perm in permutations(range(B)):
        a = ((perm[0], perm[1]), (perm[2], perm[3]))
        nq, nk = slot_counts(a)
        c = nq[0] * nk[0] + nq[1] * nk[1]
        if best is None or c < best[0]:
            best = (c, a)
    assign = best[1]
    nq, nk = slot_counts(assign)
    # slot 0 should be the smaller workload (its projections can't overlap)
    if nq[0] * nk[0] > nq[1] * nk[1]:
        assign = tuple((g[1], g[0]) for g in assign)
        nq, nk = slot_counts(assign)
    NQ, NK = nq, nk

    nc = _get_program(NQ, NK)

    # --- per-core inputs ---
    WqT = np.ascontiguousarray(Wq.T).astype(NPBF16)
    WkT = np.ascontiguousarray(Wk.T).astype(NPBF16)
    WvT = np.ascontiguousarray(Wv.T).astype(NPBF16)
    WoT = np.ascontiguousarray(Wo.T).astype(NPBF16)

    batch_qT, batch_kT, batch_vT, batch_mb, batch_kq = {}, {}, {}, {}, {}
    for b in range(B):
        batch_qT[b] = np.ascontiguousarray(query[:, b, :].T).astype(NPBF16)
        batch_kT[b] = np.ascontiguousarray(key[:, b, :].T).astype(NPBF16)
        batch_vT[b] = np.ascontiguousarray(value[:, b, :].T).astype(NPBF16)
        kpos = np.arange(S).reshape(16, 128)  # [kchunk, kpos]
        mbv = np.where(kpos >= lengths[b], np.float32(MASK_BIAS), np.float32(0.0))
        batch_mb[b] = np.ascontiguousarray(mbv.T).astype(np.float32)  # [128, 16]
        batch_kq[b] = (np.arange(S).reshape(4, 512) < lengths[b]).astype(np.float32)

    in_maps = []
    for c in range(N_CORES):
        g, hq = c // 4, c % 4
        f0 = hq * 256
        m = {
            "wqT": np.ascontiguousarray(WqT[:, f0:f0 + 256]),
            "wkT": np.ascontiguousarray(WkT[:, f0:f0 + 256]),
            "wvT": np.ascontiguousarray(WvT[:, f0:f0 + 256]),
            "woT": np.ascontiguousarray(WoT[f0:f0 + 256, :]),
        }
        for sl in range(2):
            b = assign[g][sl]
            m[f"qT{sl}"] = batch_qT[b]
            m[f"kT{sl}"] = batch_kT[b]
            m[f"vT{sl}"] = batch_vT[b]
            m[f"mb{sl}"] = batch_mb[b]
            m[f"kq{sl}"] = batch_kq[b]
        in_maps.append(m)

    res = run_bass_kernel_spmd(nc, in_maps, list(range(N_CORES)))

    # --- gather: sum 4 head-quad partials per batch, transpose ---
    out = np.zeros((S, B, H), dtype=np.float32)
    for g in range(2):
        for sl in range(2):
            b = assign[g][sl]
            acc = np.zeros((H, S), dtype=np.float32)
            for hq in range(4):
                c = g * 4 + hq
                acc += res.results[c][f"y{sl}"].astype(np.float32)
            out[:, b, :] = acc.T
    return out
